# revision 5
# baseline (speedup 1.0000x reference)
"""Trainium2 Bass kernel for nn_AnalyticalStage2.

Math (per batch element b, time index i):
    alpha = E1*E2 / ((E1+E2)*eta)
    A     = C/(E1+E2)
    D     = C*E1/(E2*(E1+E2))
    decay d = exp(-alpha * dt)   (uniform grid -> constant per b)
    v_i = d*v_{i-1} + p_i,  omega_i = A*p_i + c*v_{i-1},  c = D*(1-d)

Implementation notes:
  * Host-side marshalling: p is split into even/odd time phases per
    16384-step half, the even phase is prescaled by the per-row
    constant d, and both are cast to bf16 (DRAM layout per half:
    [d*p_even | p_odd]). The output tensor is bf16 in the same
    phase-separated layout; the host reinterleaves and casts to f32.
    This halves HBM traffic and keeps every device access contiguous.
  * 512 batch rows -> 8 cores x 64 rows. Per core the sequence is
    split into two halves on 128 partitions (partition = h*64 + b).
    Half 2 scans from zero state and is fixed up at the end with
    + c*v1e*d^i, applied per phase as geometric-in-d^2 chunks on DVE.
  * The scan is 2x-decimated: gpsimd builds Q_g = (d*p_{2g}) + p_{2g+1}
    with one tensor_add; DVE scans Q with multiplier d^2 (half the
    elements) giving V_g = v_{2g+1}. DVE also forms w_g = V_g - po_g
    (= d*v_{2g}, all-positive so no cancellation).
  * omega via two diagonal-matmul passes per phase into PSUM:
      om_{2g}   = diag(A/d) @ pe + diag(c)   @ Vshift
      om_{2g+1} = diag(A)   @ po + diag(c/d) @ w
    ACT drains PSUM -> bf16 SBUF contiguously per tile. Input DMA on
    the sync HWDGE queue, output on the ACT HWDGE queue.
"""

import numpy as np
import ml_dtypes

import concourse.bass as bass
import concourse.bacc as bacc
import concourse.mybir as mybir
from concourse.bass_utils import run_bass_kernel_spmd
from concourse.tile import TileContext

_C = 0.206756
B, NT = 512, 32768
NCORES = 8
BLOC = B // NCORES  # 64
DELTA = 0.2 / (NT - 1)

F32 = mybir.dt.float32
BF16 = mybir.dt.bfloat16
ALU = mybir.AluOpType
ACTF = mybir.ActivationFunctionType

TH = NT // 2  # per-half length 16384
PH = TH // 2  # per-phase length 8192
PW = 1024  # pairs per tile
NTILES = PH // PW  # 8
MM = 512


def build(nc):
    p_ext = nc.declare_dram_parameter("p", [BLOC, NT], BF16, isOutput=False)
    hr_ext = nc.declare_dram_parameter("h_raw", [BLOC, 3], F32, isOutput=False)
    out_ext = nc.declare_dram_parameter("out", [BLOC, NT], BF16, isOutput=True)

    # (half, phase, b, pair) views of the phase-separated layouts
    p_r = p_ext[:].rearrange("b (h e t) -> h e b t", h=2, e=2)
    out_r = out_ext[:].rearrange("b (h e t) -> h e b t", h=2, e=2)

    with TileContext(nc) as tc:
        with (
            tc.tile_pool(name="const", bufs=1) as cpool,
            tc.tile_pool(name="big", bufs=1) as bigpool,
            tc.tile_pool(name="pbf", bufs=4) as bpool,
            tc.tile_pool(name="qp", bufs=2) as qpool,
            tc.tile_pool(name="vsp", bufs=2) as vpool,
            tc.tile_pool(name="wp", bufs=2) as wpool,
            tc.tile_pool(name="st", bufs=3) as stpool,
            tc.tile_pool(name="ps", bufs=2, space="PSUM") as pspool,
        ):
            # ---- prefetch the first tiles before any setup compute ----
            hr = cpool.tile([128, 3], F32)
            nc.sync.dma_start(out=hr[0:64, :], in_=hr_ext[:])
            nc.sync.dma_start(out=hr[64:128, :], in_=hr_ext[:])

            pe_t, po_t = {}, {}
            for k in range(min(3, NTILES)):
                lp = k * PW
                pe = bpool.tile([128, PW], BF16, tag="pe")
                po = bpool.tile([128, PW], BF16, tag="po")
                nc.sync.dma_start(out=pe[0:64, :], in_=p_r[0, 0, :, lp : lp + PW])
                nc.sync.dma_start(out=pe[64:128, :], in_=p_r[1, 0, :, lp : lp + PW])
                nc.sync.dma_start(out=po[0:64, :], in_=p_r[0, 1, :, lp : lp + PW])
                nc.sync.dma_start(out=po[64:128, :], in_=p_r[1, 1, :, lp : lp + PW])
                pe_t[k], po_t[k] = pe, po

            # ---- params on all 128 rows ----
            E1, E2, eta = hr[:, 0:1], hr[:, 1:2], hr[:, 2:3]
            prm = cpool.tile([128, 24], F32)

            def pc(i):
                return prm[:, i : i + 1]

            s, se, rse, e12 = pc(0), pc(1), pc(2), pc(3)
            alpha, lnd, d, rs = pc(4), pc(5), pc(6), pc(7)
            A, rE2, t2, t3 = pc(8), pc(9), pc(10), pc(11)
            D, omd, c = pc(12), pc(13), pc(14)
            rd, Ad, crd = pc(15), pc(16), pc(17)

            nc.vector.tensor_add(out=s, in0=E1, in1=E2)
            nc.vector.tensor_mul(out=se, in0=s, in1=eta)
            nc.vector.reciprocal(rse, se)
            nc.vector.tensor_mul(out=e12, in0=E1, in1=E2)
            nc.vector.tensor_mul(out=alpha, in0=e12, in1=rse)
            nc.vector.tensor_scalar_mul(lnd, alpha, -DELTA)
            nc.scalar.activation(d, lnd, ACTF.Exp)
            nc.vector.reciprocal(rs, s)
            nc.vector.tensor_scalar_mul(A, rs, _C)
            nc.vector.reciprocal(rE2, E2)
            nc.vector.tensor_mul(out=t2, in0=E1, in1=rE2)
            nc.vector.tensor_mul(out=t3, in0=t2, in1=rs)
            nc.vector.tensor_scalar_mul(D, t3, _C)
            nc.vector.tensor_scalar(omd, d, -1.0, 1.0, ALU.mult, ALU.add)
            nc.vector.tensor_mul(out=c, in0=D, in1=omd)
            nc.vector.reciprocal(rd, d)
            nc.vector.tensor_mul(out=Ad, in0=A, in1=rd)  # A/d
            nc.vector.tensor_mul(out=crd, in0=c, in1=rd)  # c/d

            # dks[:, j] = d^(2^j), j = 0..13
            NDK = 14
            dks = cpool.tile([128, NDK], F32)
            nc.scalar.copy(out=dks[:, 0:1], in_=d)
            for j in range(1, NDK):
                nc.vector.tensor_mul(
                    out=dks[:, j : j + 1],
                    in0=dks[:, j - 1 : j],
                    in1=dks[:, j - 1 : j],
                )
            d2 = dks[:, 1:2]

            # identity mask and the four diagonal weights (bf16)
            I01 = cpool.tile([128, 128], F32)
            one = cpool.tile([128, 1], F32)
            nc.vector.memset(one[:, :], 1.0)
            nc.gpsimd.affine_select(
                out=I01[:],
                in_=one[:, 0:1].broadcast_to([128, 128]),
                pattern=[[1, 128]],
                compare_op=ALU.is_equal,
                fill=0.0,
                base=0,
                channel_multiplier=-1,
            )
            dgAd = cpool.tile([128, 128], BF16)
            dgc = cpool.tile([128, 128], BF16)
            dgA = cpool.tile([128, 128], BF16)
            dgcrd = cpool.tile([128, 128], BF16)
            nc.vector.tensor_scalar_mul(dgAd[:], I01[:], Ad)
            nc.vector.tensor_scalar_mul(dgc[:], I01[:], c)
            nc.vector.tensor_scalar_mul(dgA[:], I01[:], A)
            nc.vector.tensor_scalar_mul(dgcrd[:], I01[:], crd)

            # omega staging in tile blocks: [t0-even | t0-odd | t1-even | ...]
            om2full = bigpool.tile([128, TH], BF16)

            zcol = cpool.tile([128, 1], BF16)
            nc.vector.memset(zcol[:, :], 0.0)

            # G2[m] = (d^2)^m, m in [0, PW): built on ACT mid-loop
            G2 = cpool.tile([128, PW], F32)
            G2bf = cpool.tile([128, PW], BF16)

            # ---- streaming phase ----
            prev_vv = None
            for k in range(NTILES):
                lp = k * PW
                if k in pe_t:
                    pe, po = pe_t[k], po_t[k]
                else:
                    pe = bpool.tile([128, PW], BF16, tag="pe")
                    po = bpool.tile([128, PW], BF16, tag="po")
                    nc.sync.dma_start(out=pe[0:64, :], in_=p_r[0, 0, :, lp : lp + PW])
                    nc.sync.dma_start(
                        out=pe[64:128, :], in_=p_r[1, 0, :, lp : lp + PW]
                    )
                    nc.sync.dma_start(out=po[0:64, :], in_=p_r[0, 1, :, lp : lp + PW])
                    nc.sync.dma_start(
                        out=po[64:128, :], in_=p_r[1, 1, :, lp : lp + PW]
                    )

                # Q_g = d*p_{2g} + p_{2g+1}
                qt = qpool.tile([128, PW], BF16, tag="qt")
                nc.gpsimd.tensor_add(out=qt[:], in0=pe[:], in1=po[:])

                # vv: col 0 pad, col 1 carry V_{-1}, cols 2..PW+1 scan out
                vv = vpool.tile([128, PW + 2], BF16, tag="vv")
                init = zcol[:, 0:1] if prev_vv is None else prev_vv[:, PW + 1 : PW + 2]
                nc.vector.tensor_tensor_scan(
                    out=vv[:, 2 : PW + 2],
                    data0=d2.broadcast_to([128, PW]),
                    data1=qt[:],
                    initial=init,
                    op0=ALU.mult,
                    op1=ALU.add,
                )
                nc.scalar.copy(out=vv[:, 1:2], in_=init)
                vsh = vv[:, 1 : PW + 1]  # index g -> V_{g-1}

                # w_g = V_g - po_g  (= d*v_{2g})
                wt = wpool.tile([128, PW], BF16, tag="wt")
                nc.vector.tensor_sub(out=wt[:], in0=vv[:, 2 : PW + 2], in1=po[:])

                if k == 0:
                    # build G2 on ACT while the pipeline warms up
                    ndbl = PW.bit_length() - 1
                    nc.vector.memset(G2[:, 0:1], 1.0)
                    kk = 1
                    for j in range(ndbl):
                        nc.scalar.activation(
                            G2[:, kk : 2 * kk],
                            G2[:, 0:kk],
                            ACTF.Copy,
                            scale=dks[:, j + 1 : j + 2],
                        )
                        kk *= 2
                    nc.scalar.copy(out=G2bf[:, :], in_=G2[:, :])

                # psum: [even (PW) | odd (PW)]
                ps = pspool.tile([128, 2 * PW], F32)
                for j in range(PW // MM):
                    nc.tensor.matmul(
                        ps[:, j * MM : (j + 1) * MM],
                        dgAd[:],
                        pe[:, j * MM : (j + 1) * MM],
                        start=True,
                        stop=False,
                    )
                for j in range(PW // MM):
                    nc.tensor.matmul(
                        ps[:, j * MM : (j + 1) * MM],
                        dgc[:],
                        vsh[:, j * MM : j * MM + MM],
                        start=False,
                        stop=True,
                    )
                for j in range(PW // MM):
                    nc.tensor.matmul(
                        ps[:, PW + j * MM : PW + (j + 1) * MM],
                        dgA[:],
                        po[:, j * MM : (j + 1) * MM],
                        start=True,
                        stop=False,
                    )
                for j in range(PW // MM):
                    nc.tensor.matmul(
                        ps[:, PW + j * MM : PW + (j + 1) * MM],
                        dgcrd[:],
                        wt[:, j * MM : (j + 1) * MM],
                        start=False,
                        stop=True,
                    )

                # one contiguous drain per tile
                nc.scalar.copy(
                    out=om2full[:, 2 * lp : 2 * lp + 2 * PW], in_=ps[:]
                )
                nc.scalar.dma_start(
                    out=out_r[0, 0, :, lp : lp + PW],
                    in_=om2full[0:64, 2 * lp : 2 * lp + PW],
                )
                nc.scalar.dma_start(
                    out=out_r[0, 1, :, lp : lp + PW],
                    in_=om2full[0:64, 2 * lp + PW : 2 * lp + 2 * PW],
                )
                prev_vv = vv

            # ---- tail: half-2 fixup + c*v1e*d^i ----
            # per tile block k: even fix qE_k*(d^2)^m, odd fix qE_k*d*(d^2)^m
            v1e = cpool.tile([128, 1], F32)
            nc.gpsimd.dma_start(
                out=v1e[64:128, :], in_=prev_vv[0:64, PW + 1 : PW + 2]
            )
            qfe = cpool.tile([128, 2 * NTILES], F32)  # [qE_0..7 | qO_0..7]
            nc.vector.tensor_mul(
                out=qfe[64:128, 0:1], in0=v1e[64:128, :], in1=prm[64:128, 14:15]
            )
            kq = 1
            while kq < NTILES:
                j = 11 + kq.bit_length() - 1  # d^(2048*kq): dks[11] = d^2048
                nc.vector.tensor_scalar_mul(
                    qfe[64:128, kq : 2 * kq],
                    qfe[64:128, 0:kq],
                    dks[64:128, j : j + 1],
                )
                kq *= 2
            nc.vector.tensor_scalar_mul(
                qfe[64:128, NTILES : 2 * NTILES],
                qfe[64:128, 0:NTILES],
                dks[64:128, 0:1],
            )

            for l in range(NTILES):
                lo = 2 * l * PW
                tmp = stpool.tile([128, 2 * PW], BF16, tag="tmpbf")
                stage = stpool.tile([128, 2 * PW], BF16, tag="stage")
                nc.vector.tensor_scalar_mul(
                    tmp[64:128, 0:PW],
                    G2bf[64:128, :],
                    qfe[64:128, l : l + 1],
                )
                nc.vector.tensor_scalar_mul(
                    tmp[64:128, PW : 2 * PW],
                    G2bf[64:128, :],
                    qfe[64:128, NTILES + l : NTILES + l + 1],
                )
                nc.vector.tensor_add(
                    out=stage[64:128, :],
                    in0=tmp[64:128, :],
                    in1=om2full[64:128, lo : lo + 2 * PW],
                )
                nc.scalar.dma_start(
                    out=out_r[1, 0, :, l * PW : (l + 1) * PW],
                    in_=stage[64:128, 0:PW],
                )
                nc.scalar.dma_start(
                    out=out_r[1, 1, :, l * PW : (l + 1) * PW],
                    in_=stage[64:128, PW : 2 * PW],
                )

    return nc


def _shard(x):
    return [np.ascontiguousarray(x[i * BLOC : (i + 1) * BLOC]) for i in range(NCORES)]


def make_nc():
    nc = bacc.Bacc(None)
    build(nc)
    nc.finalize()
    return nc


def _host_d(h_raw):
    E1 = np.asarray(h_raw[:, 0], dtype=np.float64)
    E2 = np.asarray(h_raw[:, 1], dtype=np.float64)
    eta = np.asarray(h_raw[:, 2], dtype=np.float64)
    alpha = E1 * E2 / ((E1 + E2) * eta)
    return np.exp(-alpha * DELTA)[:, None]


def _preprocess_p(p, h_raw):
    """Per half: [d*p_even | p_odd], bf16."""
    p = np.asarray(p, dtype=np.float64)
    d = _host_d(h_raw)
    out = np.empty((B, NT), dtype=ml_dtypes.bfloat16)
    for h in range(2):
        src = p[:, h * TH : (h + 1) * TH]
        out[:, h * TH : h * TH + PH] = (d * src[:, 0::2]).astype(ml_dtypes.bfloat16)
        out[:, h * TH + PH : (h + 1) * TH] = src[:, 1::2].astype(ml_dtypes.bfloat16)
    return out


def _postprocess_out(raw):
    """Reinterleave [even | odd] phase blocks back to natural time order."""
    raw = np.asarray(raw, dtype=np.float32)
    out = np.empty_like(raw)
    for h in range(2):
        blk = raw[:, h * TH : (h + 1) * TH]
        out[:, h * TH : (h + 1) * TH : 2] = blk[:, 0:PH]
        out[:, h * TH + 1 : (h + 1) * TH : 2] = blk[:, PH:TH]
    return out


def run(inputs, trace=False):
    nc = make_nc()
    p_pre = _preprocess_p(inputs["p"], inputs["h_raw"])
    p_sh = _shard(p_pre)
    hr_sh = _shard(np.asarray(inputs["h_raw"], dtype=np.float32))
    in_maps = [{"p": p_sh[i], "h_raw": hr_sh[i]} for i in range(NCORES)]
    res = run_bass_kernel_spmd(nc, in_maps, core_ids=list(range(NCORES)), trace=trace)
    out = np.concatenate(
        [_postprocess_out(res.results[i]["out"]) for i in range(NCORES)], axis=0
    )
    return out, res


def kernel(h, t, p, h_raw):
    out, _ = run({"p": p, "h_raw": h_raw})
    return out


# revision 9
# speedup vs baseline: 1.0055x; 1.0055x over previous
"""Trainium2 Bass kernel for nn_AnalyticalStage2.

Math (per batch element b, time index i):
    alpha = E1*E2 / ((E1+E2)*eta)
    A     = C/(E1+E2)
    D     = C*E1/(E2*(E1+E2))
    decay d = exp(-alpha * dt)   (uniform grid -> constant per b)
    v_i = d*v_{i-1} + p_i,  omega_i = A*p_i + c*v_{i-1},  c = D*(1-d)

Implementation notes:
  * Host-side marshalling: p is split into even/odd time phases per
    16384-step half, the even phase is prescaled by the per-row
    constant d, and both are cast to bf16 (DRAM layout per half:
    [d*p_even | p_odd]). The output tensor is bf16 in the same
    phase-separated layout; the host reinterleaves and casts to f32.
    This halves HBM traffic and keeps every device access contiguous.
  * 512 batch rows -> 8 cores x 64 rows. Per core the sequence is
    split into two halves on 128 partitions (partition = h*64 + b).
    Half 2 scans from zero state and is fixed up at the end with
    + c*v1e*d^i, applied per phase as geometric-in-d^2 chunks on DVE.
  * The scan is 2x-decimated: gpsimd builds Q_g = (d*p_{2g}) + p_{2g+1}
    with one tensor_add; DVE scans Q with multiplier d^2 (half the
    elements) giving V_g = v_{2g+1}. DVE also forms w_g = V_g - po_g
    (= d*v_{2g}, all-positive so no cancellation).
  * omega via two diagonal-matmul passes per phase into PSUM:
      om_{2g}   = diag(A/d) @ pe + diag(c)   @ Vshift
      om_{2g+1} = diag(A)   @ po + diag(c/d) @ w
    ACT drains PSUM -> bf16 SBUF contiguously per tile. Input DMA on
    the sync HWDGE queue, output on the ACT HWDGE queue.
"""

import numpy as np
import ml_dtypes

import concourse.bass as bass
import concourse.bacc as bacc
import concourse.mybir as mybir
from concourse.bass_utils import run_bass_kernel_spmd
from concourse.tile import TileContext

_C = 0.206756
B, NT = 512, 32768
NCORES = 8
BLOC = B // NCORES  # 64
DELTA = 0.2 / (NT - 1)

F32 = mybir.dt.float32
BF16 = mybir.dt.bfloat16
ALU = mybir.AluOpType
ACTF = mybir.ActivationFunctionType

TH = NT // 2  # per-half length 16384
PH = TH // 2  # per-phase length 8192
PW = 1024  # pairs per tile
NTILES = PH // PW  # 8
MM = 512


def build(nc):
    p_ext = nc.declare_dram_parameter("p", [BLOC, NT], BF16, isOutput=False)
    hr_ext = nc.declare_dram_parameter("h_raw", [BLOC, 3], F32, isOutput=False)
    out_ext = nc.declare_dram_parameter("out", [BLOC, NT], BF16, isOutput=True)

    # (half, phase, b, pair) views of the phase-separated layouts
    p_r = p_ext[:].rearrange("b (h e t) -> h e b t", h=2, e=2)
    out_r = out_ext[:].rearrange("b (h e t) -> h e b t", h=2, e=2)

    with TileContext(nc) as tc:
        with (
            tc.tile_pool(name="const", bufs=1) as cpool,
            tc.tile_pool(name="big", bufs=1) as bigpool,
            tc.tile_pool(name="pbf", bufs=4) as bpool,
            tc.tile_pool(name="qp", bufs=2) as qpool,
            tc.tile_pool(name="vsp", bufs=2) as vpool,
            tc.tile_pool(name="wp", bufs=2) as wpool,
            tc.tile_pool(name="st", bufs=3) as stpool,
            tc.tile_pool(name="ps", bufs=2, space="PSUM") as pspool,
        ):
            # ---- prefetch the first tiles before any setup compute ----
            hr = cpool.tile([128, 3], F32)
            nc.sync.dma_start(out=hr[0:64, :], in_=hr_ext[:])
            nc.sync.dma_start(out=hr[64:128, :], in_=hr_ext[:])

            pe_t, po_t = {}, {}
            for k in range(min(3, NTILES)):
                lp = k * PW
                pe = bpool.tile([128, PW], BF16, tag="pe")
                po = bpool.tile([128, PW], BF16, tag="po")
                nc.sync.dma_start(out=pe[0:64, :], in_=p_r[0, 0, :, lp : lp + PW])
                nc.sync.dma_start(out=pe[64:128, :], in_=p_r[1, 0, :, lp : lp + PW])
                nc.sync.dma_start(out=po[0:64, :], in_=p_r[0, 1, :, lp : lp + PW])
                nc.sync.dma_start(out=po[64:128, :], in_=p_r[1, 1, :, lp : lp + PW])
                pe_t[k], po_t[k] = pe, po

            # ---- params: shortest path to d^2 first (gates scan 0) ----
            E1, E2, eta = hr[:, 0:1], hr[:, 1:2], hr[:, 2:3]
            prm = cpool.tile([128, 24], F32)

            def pc(i):
                return prm[:, i : i + 1]

            s, se, rse, e12 = pc(0), pc(1), pc(2), pc(3)
            alpha, lnd, d, rs = pc(4), pc(5), pc(6), pc(7)
            A, rE2, t2, t3 = pc(8), pc(9), pc(10), pc(11)
            D, omd, c = pc(12), pc(13), pc(14)
            rd, Ad, crd = pc(15), pc(16), pc(17)

            NDK = 14
            dks = cpool.tile([128, NDK], F32)
            d2 = dks[:, 1:2]

            nc.vector.tensor_add(out=s, in0=E1, in1=E2)
            nc.vector.tensor_mul(out=se, in0=s, in1=eta)
            nc.vector.reciprocal(rse, se)
            nc.vector.tensor_mul(out=e12, in0=E1, in1=E2)
            nc.vector.tensor_mul(out=alpha, in0=e12, in1=rse)
            nc.vector.tensor_scalar_mul(lnd, alpha, -DELTA)
            nc.scalar.activation(d, lnd, ACTF.Exp)
            nc.scalar.copy(out=dks[:, 0:1], in_=d)
            nc.vector.tensor_mul(out=d2, in0=d, in1=d)

            # declared here, populated inside the loop shadow (k==0)
            I01 = cpool.tile([128, 128], F32)
            one = cpool.tile([128, 1], F32)
            dgAd = cpool.tile([128, 128], BF16)
            dgc = cpool.tile([128, 128], BF16)
            dgA = cpool.tile([128, 128], BF16)
            dgcrd = cpool.tile([128, 128], BF16)

            # omega staging in tile blocks: [t0-even | t0-odd | t1-even | ...]
            om2full = bigpool.tile([128, TH], BF16)

            zcol = cpool.tile([128, 1], BF16)
            nc.vector.memset(zcol[:, :], 0.0)

            # G2[m] = (d^2)^m, m in [0, PW): built on ACT mid-loop
            G2 = cpool.tile([128, PW], F32)
            G2bf = cpool.tile([128, PW], BF16)

            # ---- streaming phase ----
            prev_vv = None
            for k in range(NTILES):
                lp = k * PW
                if k in pe_t:
                    pe, po = pe_t[k], po_t[k]
                else:
                    pe = bpool.tile([128, PW], BF16, tag="pe")
                    po = bpool.tile([128, PW], BF16, tag="po")
                    nc.sync.dma_start(out=pe[0:64, :], in_=p_r[0, 0, :, lp : lp + PW])
                    nc.sync.dma_start(
                        out=pe[64:128, :], in_=p_r[1, 0, :, lp : lp + PW]
                    )
                    nc.sync.dma_start(out=po[0:64, :], in_=p_r[0, 1, :, lp : lp + PW])
                    nc.sync.dma_start(
                        out=po[64:128, :], in_=p_r[1, 1, :, lp : lp + PW]
                    )

                # Q_g = d*p_{2g} + p_{2g+1}
                qt = qpool.tile([128, PW], BF16, tag="qt")
                nc.gpsimd.tensor_add(out=qt[:], in0=pe[:], in1=po[:])

                # vv: cols 0-2 pad (keeps the scan output 8B-aligned for
                # every pool rotation), col 3 carry V_{-1}, cols 4..PW+3 out
                vv = vpool.tile([128, PW + 4], BF16, tag="vv")
                init = zcol[:, 0:1] if prev_vv is None else prev_vv[:, PW + 3 : PW + 4]
                nc.vector.tensor_tensor_scan(
                    out=vv[:, 4 : PW + 4],
                    data0=d2.broadcast_to([128, PW]),
                    data1=qt[:],
                    initial=init,
                    op0=ALU.mult,
                    op1=ALU.add,
                )
                nc.scalar.copy(out=vv[:, 3:4], in_=init)
                vsh = vv[:, 3 : PW + 3]  # index g -> V_{g-1}

                # w_g = V_g - po_g  (= d*v_{2g})
                wt = wpool.tile([128, PW], BF16, tag="wt")
                nc.vector.tensor_sub(out=wt[:], in0=vv[:, 4 : PW + 4], in1=po[:])

                if k == 0:
                    # deferred setup, off the scan-0 critical path:
                    # identity mask (gpsimd runs it after qt_0)
                    nc.vector.memset(one[:, :], 1.0)
                    nc.gpsimd.affine_select(
                        out=I01[:],
                        in_=one[:, 0:1].broadcast_to([128, 128]),
                        pattern=[[1, 128]],
                        compare_op=ALU.is_equal,
                        fill=0.0,
                        base=0,
                        channel_multiplier=-1,
                    )
                    # remaining scalar params
                    nc.vector.reciprocal(rs, s)
                    nc.vector.tensor_scalar_mul(A, rs, _C)
                    nc.vector.reciprocal(rE2, E2)
                    nc.vector.tensor_mul(out=t2, in0=E1, in1=rE2)
                    nc.vector.tensor_mul(out=t3, in0=t2, in1=rs)
                    nc.vector.tensor_scalar_mul(D, t3, _C)
                    nc.vector.tensor_scalar(
                        omd, d, -1.0, 1.0, ALU.mult, ALU.add
                    )
                    nc.vector.tensor_mul(out=c, in0=D, in1=omd)
                    nc.vector.reciprocal(rd, d)
                    nc.vector.tensor_mul(out=Ad, in0=A, in1=rd)  # A/d
                    nc.vector.tensor_mul(out=crd, in0=c, in1=rd)  # c/d
                    nc.vector.tensor_scalar_mul(dgAd[:], I01[:], Ad)
                    nc.vector.tensor_scalar_mul(dgc[:], I01[:], c)
                    nc.vector.tensor_scalar_mul(dgA[:], I01[:], A)
                    nc.vector.tensor_scalar_mul(dgcrd[:], I01[:], crd)
                if k == 1:
                    # d^(2^j) ladder (tail-only)
                    for j in range(2, NDK):
                        nc.vector.tensor_mul(
                            out=dks[:, j : j + 1],
                            in0=dks[:, j - 1 : j],
                            in1=dks[:, j - 1 : j],
                        )
                if k == 2:
                    # G2 geometric tile on ACT (tail-only)
                    ndbl = PW.bit_length() - 1
                    nc.vector.memset(G2[:, 0:1], 1.0)
                    kk = 1
                    for j in range(ndbl):
                        nc.scalar.activation(
                            G2[:, kk : 2 * kk],
                            G2[:, 0:kk],
                            ACTF.Copy,
                            scale=dks[:, j + 1 : j + 2],
                        )
                        kk *= 2
                    nc.scalar.copy(out=G2bf[:, :], in_=G2[:, :])

                # psum: [even (PW) | odd (PW)]
                ps = pspool.tile([128, 2 * PW], F32)
                for j in range(PW // MM):
                    nc.tensor.matmul(
                        ps[:, j * MM : (j + 1) * MM],
                        dgAd[:],
                        pe[:, j * MM : (j + 1) * MM],
                        start=True,
                        stop=False,
                    )
                for j in range(PW // MM):
                    nc.tensor.matmul(
                        ps[:, j * MM : (j + 1) * MM],
                        dgc[:],
                        vsh[:, j * MM : j * MM + MM],
                        start=False,
                        stop=True,
                    )
                for j in range(PW // MM):
                    nc.tensor.matmul(
                        ps[:, PW + j * MM : PW + (j + 1) * MM],
                        dgA[:],
                        po[:, j * MM : (j + 1) * MM],
                        start=True,
                        stop=False,
                    )
                for j in range(PW // MM):
                    nc.tensor.matmul(
                        ps[:, PW + j * MM : PW + (j + 1) * MM],
                        dgcrd[:],
                        wt[:, j * MM : (j + 1) * MM],
                        start=False,
                        stop=True,
                    )

                # one contiguous drain per tile
                nc.scalar.copy(
                    out=om2full[:, 2 * lp : 2 * lp + 2 * PW], in_=ps[:]
                )
                nc.scalar.dma_start(
                    out=out_r[0, 0, :, lp : lp + PW],
                    in_=om2full[0:64, 2 * lp : 2 * lp + PW],
                )
                nc.scalar.dma_start(
                    out=out_r[0, 1, :, lp : lp + PW],
                    in_=om2full[0:64, 2 * lp + PW : 2 * lp + 2 * PW],
                )
                prev_vv = vv

            # ---- tail: half-2 fixup + c*v1e*d^i ----
            # per tile block k: even fix qE_k*(d^2)^m, odd fix qE_k*d*(d^2)^m
            v1e = cpool.tile([128, 1], F32)
            nc.gpsimd.dma_start(
                out=v1e[64:128, :], in_=prev_vv[0:64, PW + 3 : PW + 4]
            )
            qfe = cpool.tile([128, 2 * NTILES], F32)  # [qE_0..7 | qO_0..7]
            nc.vector.tensor_mul(
                out=qfe[64:128, 0:1], in0=v1e[64:128, :], in1=prm[64:128, 14:15]
            )
            kq = 1
            while kq < NTILES:
                j = 11 + kq.bit_length() - 1  # d^(2048*kq): dks[11] = d^2048
                nc.vector.tensor_scalar_mul(
                    qfe[64:128, kq : 2 * kq],
                    qfe[64:128, 0:kq],
                    dks[64:128, j : j + 1],
                )
                kq *= 2
            nc.vector.tensor_scalar_mul(
                qfe[64:128, NTILES : 2 * NTILES],
                qfe[64:128, 0:NTILES],
                dks[64:128, 0:1],
            )

            for l in range(NTILES):
                lo = 2 * l * PW
                tmp = stpool.tile([128, 2 * PW], BF16, tag="tmpbf")
                stage = stpool.tile([128, 2 * PW], BF16, tag="stage")
                nc.vector.tensor_scalar_mul(
                    tmp[64:128, 0:PW],
                    G2bf[64:128, :],
                    qfe[64:128, l : l + 1],
                )
                nc.vector.tensor_scalar_mul(
                    tmp[64:128, PW : 2 * PW],
                    G2bf[64:128, :],
                    qfe[64:128, NTILES + l : NTILES + l + 1],
                )
                nc.vector.tensor_add(
                    out=stage[64:128, :],
                    in0=tmp[64:128, :],
                    in1=om2full[64:128, lo : lo + 2 * PW],
                )
                nc.scalar.dma_start(
                    out=out_r[1, 0, :, l * PW : (l + 1) * PW],
                    in_=stage[64:128, 0:PW],
                )
                nc.scalar.dma_start(
                    out=out_r[1, 1, :, l * PW : (l + 1) * PW],
                    in_=stage[64:128, PW : 2 * PW],
                )

    return nc


def _shard(x):
    return [np.ascontiguousarray(x[i * BLOC : (i + 1) * BLOC]) for i in range(NCORES)]


def make_nc():
    nc = bacc.Bacc(None)
    build(nc)
    nc.finalize()
    return nc


def _host_d(h_raw):
    E1 = np.asarray(h_raw[:, 0], dtype=np.float64)
    E2 = np.asarray(h_raw[:, 1], dtype=np.float64)
    eta = np.asarray(h_raw[:, 2], dtype=np.float64)
    alpha = E1 * E2 / ((E1 + E2) * eta)
    return np.exp(-alpha * DELTA)[:, None]


def _preprocess_p(p, h_raw):
    """Per half: [d*p_even | p_odd], bf16."""
    p = np.asarray(p, dtype=np.float64)
    d = _host_d(h_raw)
    out = np.empty((B, NT), dtype=ml_dtypes.bfloat16)
    for h in range(2):
        src = p[:, h * TH : (h + 1) * TH]
        out[:, h * TH : h * TH + PH] = (d * src[:, 0::2]).astype(ml_dtypes.bfloat16)
        out[:, h * TH + PH : (h + 1) * TH] = src[:, 1::2].astype(ml_dtypes.bfloat16)
    return out


def _postprocess_out(raw):
    """Reinterleave [even | odd] phase blocks back to natural time order."""
    raw = np.asarray(raw, dtype=np.float32)
    out = np.empty_like(raw)
    for h in range(2):
        blk = raw[:, h * TH : (h + 1) * TH]
        out[:, h * TH : (h + 1) * TH : 2] = blk[:, 0:PH]
        out[:, h * TH + 1 : (h + 1) * TH : 2] = blk[:, PH:TH]
    return out


def run(inputs, trace=False):
    nc = make_nc()
    p_pre = _preprocess_p(inputs["p"], inputs["h_raw"])
    p_sh = _shard(p_pre)
    hr_sh = _shard(np.asarray(inputs["h_raw"], dtype=np.float32))
    in_maps = [{"p": p_sh[i], "h_raw": hr_sh[i]} for i in range(NCORES)]
    res = run_bass_kernel_spmd(nc, in_maps, core_ids=list(range(NCORES)), trace=trace)
    out = np.concatenate(
        [_postprocess_out(res.results[i]["out"]) for i in range(NCORES)], axis=0
    )
    return out, res


def kernel(h, t, p, h_raw):
    out, _ = run({"p": p, "h_raw": h_raw})
    return out
